# revision 8
# baseline (speedup 1.0000x reference)
"""Trainium2 Bass kernel for ComplexProjection:
    out[b,r,p] = |sum_s complex(x_real,x_imag)[b,r,s] * projection[r,s,p]|

Strategy: data-parallel over the particle axis B across 8 NeuronCores.
The kernel is HBM-bandwidth bound, so inputs/outputs are moved in
reduced precision (tolerance is 2e-2):

  x shipped as [r, s, {re, im}, b] fp8 e3m4     (16.8 MB per core)
  w as [s, r, p] fp16                           (0.5 MB)
  device computes ssq = re^2 + im^2, stores fp16 [r, p, b] (16.8 MB);
  the host takes the sqrt.  Measured end-to-end rel err ~9.5e-3.

Per r and 1024-wide b-chunk (two fp32 PSUM banks, 2 matmuls per
component at N=512):
    ps_re[p,c] = sum_s w[r,s,p] * x[r,s,0,c]    (PE matmul, W stationary)
    ps_im[p,c] = sum_s w[r,s,p] * x[r,s,1,c]
epilogue (GPSIMD cannot read PSUM; fp32 PSUM reads run 1x on DVE/ACT;
fp16 SBUF tensor_tensor runs 2x on DVE), rotated across chunks so
ACT/DVE/GPSIMD land ~equal busy time:
    ACT:  sq_i = ps_im^2 -> fp16 always; also sq_r on 3/8 of chunks
    DVE:  cp_r = copy(ps_re) on 5/8; mult/add split with GPSIMD
All SBUF-side epilogue math is bf16 (the DVE 2x tensor_tensor uop
exists for bf16, not fp16); the host takes the final sqrt.
"""

import os

import numpy as np

B, R, S, P = 32768, 16, 128, 128
NCORES = 8
BC = B // NCORES  # 4096 particles per core
CH = 1024         # epilogue chunk (two fp32 PSUM banks)
MMN = 512         # matmul moving dim (one bank)

XSUB = int(os.environ.get("KXSUB", "4096"))  # b-range per x DMA
NXS = BC // XSUB

XDT = os.environ.get("KXDT", "fp8e3")  # fp8e3 | fp16
WDT = os.environ.get("KWDT", "fp16")   # fp16 | fp8e3 (scaled by 16)

# (who_squares_r, who_mults, who_adds) per chunk-index mod 8.
# "A" = ACT square (no copy/mult needed), else DVE copies and the
# listed engine does the fp16 square; last slot is the add engine.
ROT = [
    ("A", None, "V"),
    ("V", "V", "G"),
    ("V", "V", "V"),
    ("A", None, "G"),
    ("V", "V", "G"),
    ("V", "G", "V"),
    ("A", None, "G"),
    ("V", "V", "G"),
]

_prog_cache = {}


def _build(nc, tile, mybir):
    f32 = mybir.dt.float32
    f16 = mybir.dt.float16
    bf16 = mybir.dt.bfloat16
    xdt = {"fp8e3": mybir.dt.float8e3, "fp16": f16}[XDT]
    wdt = {"fp8e3": mybir.dt.float8e3, "fp16": f16}[WDT]
    x = nc.dram_tensor("x", [R, S, 2, BC], xdt, kind="ExternalInput")
    w = nc.dram_tensor("w", [S, R, P], wdt, kind="ExternalInput")
    o = nc.dram_tensor("o", [R, P, BC], bf16, kind="ExternalOutput")
    x_ap, w_ap, o_ap = x.ap(), w.ap(), o.ap()

    with tile.TileContext(nc) as tc:
        with (
            tc.tile_pool(name="wp", bufs=1) as wp,
            tc.tile_pool(name="xp", bufs=int(os.environ.get("KXBUFS", "4"))) as xp,
            tc.tile_pool(name="op", bufs=int(os.environ.get("KOBUFS", "4"))) as op,
            tc.tile_pool(name="sq", bufs=6) as sqp,
            tc.tile_pool(name="ps", bufs=2, space="PSUM") as psp,
        ):
            w_sb = wp.tile([S, R, P], wdt, tag="w")
            nc.sync.dma_start(w_sb[:], w_ap[:])

            ci = 0
            for r in range(R):
                wr = w_sb[:, r, :]
                for xs in range(NXS):
                    bsl = slice(xs * XSUB, (xs + 1) * XSUB)
                    x_sb = xp.tile([S, 2, XSUB], xdt, tag="x")
                    if r == 0 and xs == 0:
                        # split the very first slab so the first matmuls
                        # start as early as possible
                        q = XSUB // 4
                        for h in range(4):
                            nc.sync.dma_start(
                                x_sb[:, :, h * q:(h + 1) * q],
                                x_ap[r, :, :, h * q:(h + 1) * q])
                    else:
                        nc.sync.dma_start(x_sb[:], x_ap[r, :, :, bsl])
                    out_sb = op.tile([P, XSUB], bf16, tag="o")
                    for cc in range(XSUB // CH):
                        sl = slice(cc * CH, (cc + 1) * CH)
                        sqr_e, mul_e, add_e = ROT[ci % len(ROT)]
                        ci += 1
                        ps_r = psp.tile([P, CH], f32, tag="psr")
                        ps_i = psp.tile([P, CH], f32, tag="psi")
                        for m in range(CH // MMN):
                            msl = slice(m * MMN, (m + 1) * MMN)
                            xin = x_sb[:, :, sl]
                            nc.tensor.matmul(ps_r[:, msl], wr, xin[:, 0, msl],
                                             start=True, stop=True)
                            nc.tensor.matmul(ps_i[:, msl], wr, xin[:, 1, msl],
                                             start=True, stop=True)
                        sq_i = sqp.tile([P, CH], bf16, tag="sqi")
                        nc.scalar.square(sq_i[:], ps_i[:])
                        sq_r = sqp.tile([P, CH], bf16, tag="sqr")
                        if sqr_e == "A":
                            nc.scalar.square(sq_r[:], ps_r[:])
                        else:
                            cp_r = sqp.tile([P, CH], bf16, tag="cpr")
                            nc.vector.tensor_copy(cp_r[:], ps_r[:])
                            eng = nc.vector if mul_e == "V" else nc.gpsimd
                            eng.tensor_mul(sq_r[:], cp_r[:], cp_r[:])
                        eng = nc.vector if add_e == "V" else nc.gpsimd
                        eng.tensor_add(out_sb[:, sl], sq_r[:], sq_i[:])
                    if r == R - 1 and xs == NXS - 1:
                        # finer stores at the tail so the last compute
                        # overlaps its own writeback
                        h4 = XSUB // 4
                        for h in range(4):
                            nc.scalar.dma_start(
                                o_ap[r, :, xs * XSUB + h * h4:
                                     xs * XSUB + (h + 1) * h4],
                                out_sb[:, h * h4:(h + 1) * h4])
                    else:
                        nc.scalar.dma_start(o_ap[r, :, bsl], out_sb[:])


def _build_program():
    key = (XDT, WDT, XSUB)
    if key in _prog_cache:
        return _prog_cache[key]

    import concourse.tile as tile
    from concourse import bacc, mybir

    nc = bacc.Bacc("TRN2", target_bir_lowering=False, debug=False,
                   num_devices=NCORES)
    _build(nc, tile, mybir)
    nc.compile()
    _prog_cache[key] = nc
    return nc


LAST_RESULT = None


def kernel(x_real, x_imag, projection):
    global LAST_RESULT
    import ml_dtypes
    from concourse.bass_utils import run_bass_kernel_spmd

    nc = _build_program()

    xdt = {"fp8e3": ml_dtypes.float8_e3m4, "fp16": np.float16}[XDT]
    w32 = np.ascontiguousarray(
        np.asarray(projection, dtype=np.float32).transpose(1, 0, 2))
    if WDT == "fp16":
        w = w32.astype(np.float16)
        oscale = 1.0
    else:
        w = (w32 * 16.0).astype(ml_dtypes.float8_e3m4)
        oscale = 1.0 / 16.0

    # x: (B, R, S) re/im fp32 -> [R, S, 2, B], sliced per core on b
    xt = np.empty((R, S, 2, B), dtype=xdt)
    xt[:, :, 0, :] = np.asarray(x_real, dtype=np.float32).transpose(1, 2, 0)
    xt[:, :, 1, :] = np.asarray(x_imag, dtype=np.float32).transpose(1, 2, 0)

    in_maps = []
    for c in range(NCORES):
        sl = slice(c * BC, (c + 1) * BC)
        in_maps.append({"x": np.ascontiguousarray(xt[:, :, :, sl]), "w": w})

    res = run_bass_kernel_spmd(nc, in_maps, core_ids=list(range(NCORES)))
    LAST_RESULT = res
    out = np.empty((B, R, P), dtype=np.float32)
    for c in range(NCORES):
        ssq = res.results[c]["o"].astype(np.float32)  # [R, P, BC]
        out[c * BC:(c + 1) * BC] = oscale * np.sqrt(ssq).transpose(2, 0, 1)
    return out
